# revision 2
# baseline (speedup 1.0000x reference)
"""Trainium2 Bass kernel for the Connectome problem (segment_reduce).

Pipeline per batch element n (one NeuronCore each, data-parallel over N=8):
  latent[t,r] = mean over voxels v with label[v]==r of imgs[n,t,v]
  x = centered+L2-normalized latent over T; conn = x^T x; return triu(conn), latent.

Strategy:
  * parc_weight is a row-normalized one-hot (segment mean). Host derives
    labels/counts, sorts ROIs by segment length and permutes imgs columns so
    each ROI's voxels are contiguous and equal-length ROIs are adjacent.
  * Device (per core): stream imgs [512,16384] f32 in 4 tiles of [128,16384];
    VectorE does one grouped reduce per distinct segment length
    ([128, n_g, L] -> [128, n_g]); TensorE accumulates conn_raw = segsum^T @
    segsum and column sums, then applies the -T*mean*mean^T rank-1 correction
    via a K=1 matmul. Outputs: segsum [512,400], conn_raw (centered gram,
    diag = sum-of-squares) [400,400].
  * Host: latent = segsum/count, f = w/(w*sqrt(diag)+eps), conn = conn_raw *
    outer(f,f), unpermute ROIs, extract upper triangle.
"""

import numpy as np
from contextlib import ExitStack

N, T, HH, WW = 8, 512, 128, 128
V = HH * WW
R = 400
EPS = 1e-6
PT = 128           # partitions per tile
KT = T // PT       # 4 T-tiles
NCORES = 8

_cache = {}


def _segment_structure(parc_weight):
    """Derive labels/counts and the sorted-by-length column permutation."""
    pw = np.asarray(parc_weight)
    labels = pw.argmax(axis=0)                       # [V]
    counts = np.bincount(labels, minlength=R)        # [R]
    # verify parc_weight really is the row-normalized one-hot we assume
    expect = np.zeros_like(pw)
    expect[labels, np.arange(V)] = (np.float32(1.0) / counts[labels].astype(np.float32))
    ok = bool(np.allclose(pw, expect, rtol=0, atol=1e-7)) and counts.min() > 0
    order = np.argsort(counts, kind="stable")        # ROI ids sorted by length
    roi_rank = np.empty(R, dtype=np.int64)
    roi_rank[order] = np.arange(R)
    perm = np.argsort(roi_rank[labels], kind="stable")   # voxel permutation
    sorted_counts = counts[order]
    # run-length encode sorted_counts -> groups of (L, n_g)
    groups = []
    i = 0
    while i < R:
        j = i
        while j < R and sorted_counts[j] == sorted_counts[i]:
            j += 1
        groups.append((int(sorted_counts[i]), j - i))
        i = j
    return ok, labels, counts, order, perm, groups


def _build_program(groups):
    import concourse.tile as tile
    from concourse import bacc, mybir

    F32 = mybir.dt.float32
    nc = bacc.Bacc("TRN2", target_bir_lowering=False, debug=False, num_devices=NCORES)
    x = nc.dram_tensor("x", [T, V], F32, kind="ExternalInput").ap()
    segsum = nc.dram_tensor("segsum", [T, R], F32, kind="ExternalOutput").ap()
    conn = nc.dram_tensor("conn", [R, R], F32, kind="ExternalOutput").ap()

    with tile.TileContext(nc) as tc, ExitStack() as ctx:
        xin = ctx.enter_context(tc.tile_pool(name="xin", bufs=2))
        segp = ctx.enter_context(tc.tile_pool(name="seg", bufs=1))
        constp = ctx.enter_context(tc.tile_pool(name="const", bufs=1))
        smallp = ctx.enter_context(tc.tile_pool(name="small", bufs=1))
        psum = ctx.enter_context(tc.tile_pool(name="psum", bufs=1, space="PSUM"))
        outp = ctx.enter_context(tc.tile_pool(name="out", bufs=2))

        ones = constp.tile([PT, 1], F32, tag="ones")
        nc.vector.memset(ones[:], 1.0)

        conn_ps = [psum.tile([PT, R], F32, tag=f"conn{m}", name=f"conn_ps{m}") for m in range(4)]
        sums_ps = psum.tile([1, R], F32, tag="sums", name="sums_ps")
        seg_tiles = [segp.tile([PT, R], F32, tag=f"seg{k}", name=f"seg{k}") for k in range(KT)]

        for k in range(KT):
            xt = xin.tile([PT, V], F32, tag="xt")
            nc.sync.dma_start(xt[:], x[k * PT:(k + 1) * PT, :])
            st = seg_tiles[k]
            off = 0
            col = 0
            for (L, n_g) in groups:
                src = xt[:, off:off + n_g * L].rearrange("p (n l) -> p n l", l=L)
                nc.vector.reduce_sum(st[:, col:col + n_g], src,
                                     axis=mybir.AxisListType.X)
                off += n_g * L
                col += n_g
            nc.sync.dma_start(segsum[k * PT:(k + 1) * PT, :], st[:])
            for m in range(4):
                ms = min(PT, R - m * PT)
                nc.tensor.matmul(conn_ps[m][:ms, :], st[:, m * PT:m * PT + ms],
                                 st[:, :], start=(k == 0), stop=False)
            nc.tensor.matmul(sums_ps[:, :], ones[:, :], st[:, :],
                             start=(k == 0), stop=(k == KT - 1))

        sums_sb = smallp.tile([1, R], F32, tag="sums_sb")
        nc.scalar.mul(sums_sb[:], sums_ps[:], 1.0)
        sums_neg = smallp.tile([1, R], F32, tag="sums_neg")
        nc.scalar.mul(sums_neg[:], sums_ps[:], -1.0 / T)
        for m in range(4):
            ms = min(PT, R - m * PT)
            nc.tensor.matmul(conn_ps[m][:ms, :], sums_neg[:, m * PT:m * PT + ms],
                             sums_sb[:, :], start=False, stop=True)
        for m in range(4):
            ms = min(PT, R - m * PT)
            ct = outp.tile([PT, R], F32, tag="connout")
            nc.scalar.mul(ct[:ms, :], conn_ps[m][:ms, :], 1.0)
            nc.sync.dma_start(conn[m * PT:m * PT + ms, :], ct[:ms, :])

    nc.compile()
    return nc


def _reference_fallback(imgs, parc_weight):
    """Pure-numpy reference path, used only if parc_weight is not one-hot."""
    lat = imgs.reshape(N, T, V).astype(np.float32) @ parc_weight.T.astype(np.float32)
    x = lat - lat.mean(axis=1, keepdims=True)
    x = x / (np.linalg.norm(x, axis=1, keepdims=True) + np.float32(EPS))
    conn = np.einsum("ntr,nts->nrs", x, x)
    row, col = np.triu_indices(R, k=1)
    return conn[:, row, col][:, None, :].astype(np.float32), lat.astype(np.float32)


def kernel(imgs, parc_weight):
    imgs = np.asarray(imgs)
    parc_weight = np.asarray(parc_weight, dtype=np.float32)
    ok, labels, counts, order, perm, groups = _segment_structure(parc_weight)
    if not ok:
        return _reference_fallback(imgs, parc_weight)

    key = tuple(groups)
    if key not in _cache:
        _cache[key] = _build_program(groups)
    nc = _cache[key]

    from concourse.bass_utils import run_bass_kernel_spmd

    imgs_flat = imgs.reshape(N, T, V)
    imgs_perm = np.take(imgs_flat, perm, axis=2)       # [8, 512, 16384]
    in_maps = [{"x": np.ascontiguousarray(imgs_perm[n])} for n in range(N)]
    res = run_bass_kernel_spmd(nc, in_maps, core_ids=list(range(NCORES)))

    segsum = np.stack([r["segsum"] for r in res.results])   # [8, 512, 400] sorted order
    conn_raw = np.stack([r["conn"] for r in res.results])   # [8, 400, 400] sorted order

    w = np.float32(1.0) / counts[order].astype(np.float32)  # [400]
    latent_s = segsum * w[None, None, :]
    latent = np.empty_like(latent_s)
    latent[:, :, order] = latent_s                          # unpermute ROIs

    diag = conn_raw[:, np.arange(R), np.arange(R)]          # [8, 400] = ||xc||^2
    nrm = np.sqrt(np.maximum(diag, 0.0, dtype=np.float64)).astype(np.float32)
    f = w[None, :] / (w[None, :] * nrm + np.float32(EPS))   # [8, 400]
    conn_n = conn_raw * f[:, :, None] * f[:, None, :]
    conn_full = np.empty_like(conn_n)
    conn_full[:, order[:, None], order[None, :]] = conn_n   # unpermute both axes

    row, col = np.triu_indices(R, k=1)
    conn_vec = conn_full[:, row, col]                       # [8, 79800]
    cls_token = conn_vec[:, None, :].astype(np.float32)
    return cls_token, latent.astype(np.float32)


# revision 3
# speedup vs baseline: 1.2231x; 1.2231x over previous
"""Trainium2 Bass kernel for the Connectome problem (segment_reduce).

Pipeline per batch element n (one NeuronCore each, data-parallel over N=8):
  latent[t,r] = mean over voxels v with label[v]==r of imgs[n,t,v]
  x = centered+L2-normalized latent over T; conn = x^T x; return triu(conn), latent.

Strategy:
  * parc_weight is a row-normalized one-hot (segment mean). Host derives
    labels/counts, sorts ROIs by segment length and permutes imgs columns so
    each ROI's voxels are contiguous and equal-length ROIs are adjacent.
  * Device (per core): stream imgs [512,16384] f32 as 4 row-tiles x ~4 column
    chunks (chunk edges on ROI-group boundaries); VectorE does one grouped
    reduce per distinct segment length ([128, n_g, L] -> [128, n_g]); TensorE
    accumulates the uncentered gram conn_raw = segsum^T @ segsum in 8 PSUM
    banks (4 row-chunks x 2 col-halves). Outputs: segsum [512,400] and
    conn_raw [400,400] per core.
  * Host (float64 where it helps): center via conn_c = conn_raw -
    outer(sums,sums)/T with sums from segsum; latent = segsum/count;
    f = w/(w*sqrt(diag(conn_c))+eps); conn = conn_c * outer(f,f); unpermute
    ROIs; extract upper triangle.
"""

import numpy as np
from contextlib import ExitStack

N, T, HH, WW = 8, 512, 128, 128
V = HH * WW
R = 400
EPS = 1e-6
PT = 128           # partitions per tile
KT = T // PT       # 4 T-tiles
NCORES = 8
CHUNK_COLS = 4096  # target input-chunk width (columns)
SS = 2             # conn matmul split along output columns
SW = (R + SS - 1) // SS

_cache = {}


def _segment_structure(parc_weight):
    """Derive labels/counts and the sorted-by-length column permutation."""
    pw = np.asarray(parc_weight)
    labels = pw.argmax(axis=0)                       # [V]
    counts = np.bincount(labels, minlength=R)        # [R]
    # verify parc_weight really is the row-normalized one-hot we assume
    expect = np.zeros_like(pw)
    expect[labels, np.arange(V)] = (np.float32(1.0) / counts[labels].astype(np.float32))
    ok = bool(np.allclose(pw, expect, rtol=0, atol=1e-7)) and counts.min() > 0
    order = np.argsort(counts, kind="stable")        # ROI ids sorted by length
    roi_rank = np.empty(R, dtype=np.int64)
    roi_rank[order] = np.arange(R)
    perm = np.argsort(roi_rank[labels], kind="stable")   # voxel permutation
    sorted_counts = counts[order]
    # run-length encode sorted_counts -> groups of (L, n_g)
    groups = []
    i = 0
    while i < R:
        j = i
        while j < R and sorted_counts[j] == sorted_counts[i]:
            j += 1
        groups.append((int(sorted_counts[i]), j - i))
        i = j
    return ok, labels, counts, order, perm, groups


def _make_chunks(groups):
    """Pack groups into input-DMA chunks of ~CHUNK_COLS columns.

    Returns a list of chunks; each chunk is (col_off, width, [(L, n_g,
    seg_col)]) where col_off/width are voxel-column offsets into the permuted
    imgs row and seg_col is the output column of the group's first ROI."""
    chunks = []
    cur = []
    cur_off = 0
    off = 0
    seg_col = 0
    for (L, n_g) in groups:
        cur.append((L, n_g, seg_col))
        off += L * n_g
        seg_col += n_g
        if off - cur_off >= CHUNK_COLS:
            chunks.append((cur_off, off - cur_off, cur))
            cur = []
            cur_off = off
    if cur:
        chunks.append((cur_off, off - cur_off, cur))
    return chunks


def _build_program(groups):
    import concourse.tile as tile
    from concourse import bacc, mybir

    F32 = mybir.dt.float32
    nc = bacc.Bacc("TRN2", target_bir_lowering=False, debug=False, num_devices=NCORES)
    x = nc.dram_tensor("x", [T, V], F32, kind="ExternalInput").ap()
    segsum = nc.dram_tensor("segsum", [T, R], F32, kind="ExternalOutput").ap()
    conn = nc.dram_tensor("conn", [R, R], F32, kind="ExternalOutput").ap()

    chunks = _make_chunks(groups)
    max_w = max(w for _, w, _ in chunks)

    with tile.TileContext(nc) as tc, ExitStack() as ctx:
        xin = ctx.enter_context(tc.tile_pool(name="xin", bufs=4))
        segp = ctx.enter_context(tc.tile_pool(name="seg", bufs=1))
        psum = ctx.enter_context(tc.tile_pool(name="psum", bufs=1, space="PSUM"))
        outp = ctx.enter_context(tc.tile_pool(name="out", bufs=2))

        conn_ps = [[psum.tile([PT, SW], F32, tag=f"conn{m}s{s}",
                              name=f"conn_ps{m}_{s}") for s in range(SS)]
                   for m in range(4)]
        seg_tiles = [segp.tile([PT, R], F32, tag=f"seg{k}", name=f"seg{k}")
                     for k in range(KT)]

        for k in range(KT):
            st = seg_tiles[k]
            for (coff, cw, cgroups) in chunks:
                xt = xin.tile([PT, max_w], F32, tag="xt", name=f"xt{k}_{coff}")
                nc.sync.dma_start(xt[:, :cw], x[k * PT:(k + 1) * PT,
                                                coff:coff + cw])
                loc = 0
                for (L, n_g, seg_col) in cgroups:
                    src = xt[:, loc:loc + n_g * L].rearrange(
                        "p (n l) -> p n l", l=L)
                    nc.vector.reduce_sum(st[:, seg_col:seg_col + n_g], src,
                                         axis=mybir.AxisListType.X)
                    loc += n_g * L
            nc.sync.dma_start(segsum[k * PT:(k + 1) * PT, :], st[:])
            for m in range(4):
                ms = min(PT, R - m * PT)
                for s in range(SS):
                    sw = min(SW, R - s * SW)
                    nc.tensor.matmul(conn_ps[m][s][:ms, :sw],
                                     st[:, m * PT:m * PT + ms],
                                     st[:, s * SW:s * SW + sw],
                                     start=(k == 0), stop=(k == KT - 1))
        for m in range(4):
            ms = min(PT, R - m * PT)
            ct = outp.tile([PT, R], F32, tag="connout", name=f"connout{m}")
            for s in range(SS):
                sw = min(SW, R - s * SW)
                nc.scalar.mul(ct[:ms, s * SW:s * SW + sw],
                              conn_ps[m][s][:ms, :sw], 1.0)
            nc.sync.dma_start(conn[m * PT:m * PT + ms, :], ct[:ms, :])

    nc.compile()
    return nc


def _reference_fallback(imgs, parc_weight):
    """Pure-numpy reference path, used only if parc_weight is not one-hot."""
    lat = imgs.reshape(N, T, V).astype(np.float32) @ parc_weight.T.astype(np.float32)
    x = lat - lat.mean(axis=1, keepdims=True)
    x = x / (np.linalg.norm(x, axis=1, keepdims=True) + np.float32(EPS))
    conn = np.einsum("ntr,nts->nrs", x, x)
    row, col = np.triu_indices(R, k=1)
    return conn[:, row, col][:, None, :].astype(np.float32), lat.astype(np.float32)


def kernel(imgs, parc_weight):
    imgs = np.asarray(imgs)
    parc_weight = np.asarray(parc_weight, dtype=np.float32)
    ok, labels, counts, order, perm, groups = _segment_structure(parc_weight)
    if not ok:
        return _reference_fallback(imgs, parc_weight)

    key = tuple(groups)
    if key not in _cache:
        _cache[key] = _build_program(groups)
    nc = _cache[key]

    from concourse.bass_utils import run_bass_kernel_spmd

    imgs_flat = imgs.reshape(N, T, V)
    imgs_perm = np.take(imgs_flat, perm, axis=2)       # [8, 512, 16384]
    in_maps = [{"x": np.ascontiguousarray(imgs_perm[n])} for n in range(N)]
    res = run_bass_kernel_spmd(nc, in_maps, core_ids=list(range(NCORES)))

    segsum = np.stack([r["segsum"] for r in res.results])   # [8, 512, 400] sorted order
    conn_raw = np.stack([r["conn"] for r in res.results])   # [8, 400, 400] sorted order

    w = np.float32(1.0) / counts[order].astype(np.float32)  # [400]
    latent_s = segsum * w[None, None, :]
    latent = np.empty_like(latent_s)
    latent[:, :, order] = latent_s                          # unpermute ROIs

    # center on host in float64: conn_c = conn_raw - outer(sums, sums)/T
    sums = segsum.sum(axis=1, dtype=np.float64)             # [8, 400]
    conn_c = conn_raw.astype(np.float64) - \
        sums[:, :, None] * sums[:, None, :] * (1.0 / T)

    diag = conn_c[:, np.arange(R), np.arange(R)]            # [8, 400] = ||xc||^2
    nrm = np.sqrt(np.maximum(diag, 0.0)).astype(np.float32)
    f = w[None, :] / (w[None, :] * nrm + np.float32(EPS))   # [8, 400]
    conn_n = (conn_c * f[:, :, None].astype(np.float64)
              * f[:, None, :].astype(np.float64)).astype(np.float32)
    conn_full = np.empty_like(conn_n)
    conn_full[:, order[:, None], order[None, :]] = conn_n   # unpermute both axes

    row, col = np.triu_indices(R, k=1)
    conn_vec = conn_full[:, row, col]                       # [8, 79800]
    cls_token = conn_vec[:, None, :].astype(np.float32)
    return cls_token, latent.astype(np.float32)


# revision 6
# speedup vs baseline: 1.2779x; 1.0448x over previous
"""Trainium2 Bass kernel for the Connectome problem (segment_reduce).

Pipeline per batch element n (one NeuronCore each, data-parallel over N=8):
  latent[t,r] = mean over voxels v with label[v]==r of imgs[n,t,v]
  x = centered+L2-normalized latent over T; conn = x^T x; return triu(conn), latent.

Strategy:
  * parc_weight is a row-normalized one-hot (segment mean). Host derives
    labels/counts, sorts ROIs by segment length and permutes imgs columns so
    each ROI's voxels are contiguous and equal-length ROIs are adjacent.
  * Device (per core): stream imgs [512,16384] f32 as 4 row-tiles x ~4 column
    chunks (chunk edges on ROI-group boundaries); VectorE does one grouped
    reduce per distinct segment length ([128, n_g, L] -> [128, n_g]); TensorE
    accumulates the uncentered gram conn_raw = segsum^T @ segsum in 8 PSUM
    banks (4 row-chunks x 2 col-halves). Outputs: segsum [512,400] and
    conn_raw [400,400] per core.
  * Host (float64 where it helps): center via conn_c = conn_raw -
    outer(sums,sums)/T with sums from segsum; latent = segsum/count;
    f = w/(w*sqrt(diag(conn_c))+eps); conn = conn_c * outer(f,f); unpermute
    ROIs; extract upper triangle.
"""

import numpy as np
from contextlib import ExitStack

N, T, HH, WW = 8, 512, 128, 128
V = HH * WW
R = 400
EPS = 1e-6
PT = 128           # partitions per tile
KT = T // PT       # 4 T-tiles
NCORES = 8
CHUNK_COLS = 2048  # target input-chunk width (columns)
MCH = [(0, 128), (128, 128), (256, 128), (384, 16)]  # 128-aligned ROI chunks

_cache = {}


def _segment_structure(parc_weight):
    """Derive labels/counts and the sorted-by-length column permutation."""
    pw = np.asarray(parc_weight)
    labels = pw.argmax(axis=0)                       # [V]
    counts = np.bincount(labels, minlength=R)        # [R]
    # verify parc_weight really is the row-normalized one-hot we assume
    expect = np.zeros_like(pw)
    expect[labels, np.arange(V)] = (np.float32(1.0) / counts[labels].astype(np.float32))
    ok = bool(np.allclose(pw, expect, rtol=0, atol=1e-7)) and counts.min() > 0
    order = np.argsort(counts, kind="stable")        # ROI ids sorted by length
    roi_rank = np.empty(R, dtype=np.int64)
    roi_rank[order] = np.arange(R)
    perm = np.argsort(roi_rank[labels], kind="stable")   # voxel permutation
    sorted_counts = counts[order]
    # run-length encode sorted_counts -> groups of (L, n_g)
    groups = []
    i = 0
    while i < R:
        j = i
        while j < R and sorted_counts[j] == sorted_counts[i]:
            j += 1
        groups.append((int(sorted_counts[i]), j - i))
        i = j
    return ok, labels, counts, order, perm, groups


def _make_chunks(groups):
    """Pack groups into input-DMA chunks of ~CHUNK_COLS columns.

    Returns a list of chunks; each chunk is (col_off, width, [(L, n_g,
    seg_col)]) where col_off/width are voxel-column offsets into the permuted
    imgs row and seg_col is the output column of the group's first ROI."""
    chunks = []
    cur = []
    cur_off = 0
    off = 0
    seg_col = 0
    for (L, n_g) in groups:
        cur.append((L, n_g, seg_col))
        off += L * n_g
        seg_col += n_g
        if off - cur_off >= CHUNK_COLS:
            chunks.append((cur_off, off - cur_off, cur))
            cur = []
            cur_off = off
    if cur:
        chunks.append((cur_off, off - cur_off, cur))
    return chunks


def _build_program(groups):
    import concourse.tile as tile
    from concourse import bacc, mybir

    F32 = mybir.dt.float32
    nc = bacc.Bacc("TRN2", target_bir_lowering=False, debug=False, num_devices=NCORES)
    x = nc.dram_tensor("x", [T, V], F32, kind="ExternalInput").ap()
    segsum = nc.dram_tensor("segsum", [T, R], F32, kind="ExternalOutput").ap()
    conn = nc.dram_tensor("conn", [R, R], F32, kind="ExternalOutput").ap()

    chunks = _make_chunks(groups)
    max_w = max(w for _, w, _ in chunks)

    with tile.TileContext(nc) as tc, ExitStack() as ctx:
        xin = ctx.enter_context(tc.tile_pool(name="xin", bufs=4))
        segp = ctx.enter_context(tc.tile_pool(name="seg", bufs=1))
        psum = ctx.enter_context(tc.tile_pool(name="psum", bufs=1, space="PSUM"))
        outp = ctx.enter_context(tc.tile_pool(name="out", bufs=2))

        conn_ps = [psum.tile([PT, R], F32, tag=f"conn{m}", name=f"conn_ps{m}")
                   for m in range(4)]
        seg_tiles = [segp.tile([PT, R], F32, tag=f"seg{k}", name=f"seg{k}")
                     for k in range(KT)]

        for k in range(KT):
            st = seg_tiles[k]
            for (coff, cw, cgroups) in chunks:
                xt = xin.tile([PT, max_w], F32, tag="xt", name=f"xt{k}_{coff}")
                nc.sync.dma_start(xt[:, :cw], x[k * PT:(k + 1) * PT,
                                                coff:coff + cw])
                loc = 0
                for (L, n_g, seg_col) in cgroups:
                    src = xt[:, loc:loc + n_g * L].rearrange(
                        "p (n l) -> p n l", l=L)
                    nc.vector.reduce_sum(st[:, seg_col:seg_col + n_g], src,
                                         axis=mybir.AxisListType.X)
                    loc += n_g * L
            nc.sync.dma_start(segsum[k * PT:(k + 1) * PT, :], st[:])
            # symmetric gram: only upper blocks (m <= s), s-outer so each
            # matmul issues as soon as the columns it reads are reduced.
            # Bank m is cleared by its first matmul (k==0, s==m); later
            # blocks overwrite-or-accumulate via per-element has_written.
            for (soff, sw) in MCH:
                for (moff, ms) in MCH:
                    if moff > soff:
                        continue
                    nc.tensor.matmul(conn_ps[moff // PT][:ms, soff:soff + sw],
                                     st[:, moff:moff + ms],
                                     st[:, soff:soff + sw],
                                     start=(k == 0 and moff == soff),
                                     stop=(k == KT - 1))
        for (moff, ms) in MCH:
            uw = R - moff                       # valid (upper) width for row m
            ct = outp.tile([PT, R], F32, tag="connout", name=f"connout{moff}")
            nc.scalar.mul(ct[:ms, :uw], conn_ps[moff // PT][:ms, moff:], 1.0)
            nc.sync.dma_start(conn[moff:moff + ms, moff:], ct[:ms, :uw])

    nc.compile()
    return nc


def _reference_fallback(imgs, parc_weight):
    """Pure-numpy reference path, used only if parc_weight is not one-hot."""
    lat = imgs.reshape(N, T, V).astype(np.float32) @ parc_weight.T.astype(np.float32)
    x = lat - lat.mean(axis=1, keepdims=True)
    x = x / (np.linalg.norm(x, axis=1, keepdims=True) + np.float32(EPS))
    conn = np.einsum("ntr,nts->nrs", x, x)
    row, col = np.triu_indices(R, k=1)
    return conn[:, row, col][:, None, :].astype(np.float32), lat.astype(np.float32)


def kernel(imgs, parc_weight):
    imgs = np.asarray(imgs)
    parc_weight = np.asarray(parc_weight, dtype=np.float32)
    ok, labels, counts, order, perm, groups = _segment_structure(parc_weight)
    if not ok:
        return _reference_fallback(imgs, parc_weight)

    key = tuple(groups)
    if key not in _cache:
        _cache[key] = _build_program(groups)
    nc = _cache[key]

    from concourse.bass_utils import run_bass_kernel_spmd

    imgs_flat = imgs.reshape(N, T, V)
    imgs_perm = np.take(imgs_flat, perm, axis=2)       # [8, 512, 16384]
    in_maps = [{"x": np.ascontiguousarray(imgs_perm[n])} for n in range(N)]
    res = run_bass_kernel_spmd(nc, in_maps, core_ids=list(range(NCORES)))

    segsum = np.stack([r["segsum"] for r in res.results])   # [8, 512, 400] sorted order
    conn_raw = np.stack([r["conn"] for r in res.results])   # [8, 400, 400] sorted order
    # device computed only upper 128-aligned blocks; mirror into lower blocks
    for (moff, ms) in MCH:
        for (soff, sw) in MCH:
            if soff > moff:
                conn_raw[:, soff:soff + sw, moff:moff + ms] = \
                    conn_raw[:, moff:moff + ms, soff:soff + sw].transpose(0, 2, 1)

    w = np.float32(1.0) / counts[order].astype(np.float32)  # [400]
    latent_s = segsum * w[None, None, :]
    latent = np.empty_like(latent_s)
    latent[:, :, order] = latent_s                          # unpermute ROIs

    # center on host in float64: conn_c = conn_raw - outer(sums, sums)/T
    sums = segsum.sum(axis=1, dtype=np.float64)             # [8, 400]
    conn_c = conn_raw.astype(np.float64) - \
        sums[:, :, None] * sums[:, None, :] * (1.0 / T)

    diag = conn_c[:, np.arange(R), np.arange(R)]            # [8, 400] = ||xc||^2
    nrm = np.sqrt(np.maximum(diag, 0.0)).astype(np.float32)
    f = w[None, :] / (w[None, :] * nrm + np.float32(EPS))   # [8, 400]
    conn_n = (conn_c * f[:, :, None].astype(np.float64)
              * f[:, None, :].astype(np.float64)).astype(np.float32)
    conn_full = np.empty_like(conn_n)
    conn_full[:, order[:, None], order[None, :]] = conn_n   # unpermute both axes

    row, col = np.triu_indices(R, k=1)
    conn_vec = conn_full[:, row, col]                       # [8, 79800]
    cls_token = conn_vec[:, None, :].astype(np.float32)
    return cls_token, latent.astype(np.float32)


# revision 15
# speedup vs baseline: 1.3179x; 1.0313x over previous
"""Trainium2 Bass kernel for the Connectome problem (segment_reduce).

Pipeline per batch element n (one NeuronCore each, data-parallel over N=8):
  latent[t,r] = mean over voxels v with label[v]==r of imgs[n,t,v]
  x = centered+L2-normalized latent over T; conn = x^T x; return triu(conn), latent.

Strategy:
  * parc_weight is a row-normalized one-hot (segment mean). Host derives
    labels/counts, sorts ROIs by segment length and permutes imgs columns so
    each ROI's voxels are contiguous and equal-length ROIs are adjacent.
  * Device (per core): stream imgs [512,16384] f32 as 4 row-tiles x ~4 column
    chunks (chunk edges on ROI-group boundaries); VectorE does one grouped
    reduce per distinct segment length ([128, n_g, L] -> [128, n_g]); TensorE
    accumulates the uncentered gram conn_raw = segsum^T @ segsum in 8 PSUM
    banks (4 row-chunks x 2 col-halves). Outputs: segsum [512,400] and
    conn_raw [400,400] per core.
  * Host (float64 where it helps): center via conn_c = conn_raw -
    outer(sums,sums)/T with sums from segsum; latent = segsum/count;
    f = w/(w*sqrt(diag(conn_c))+eps); conn = conn_c * outer(f,f); unpermute
    ROIs; extract upper triangle.
"""

import numpy as np
from contextlib import ExitStack

N, T, HH, WW = 8, 512, 128, 128
V = HH * WW
R = 400
EPS = 1e-6
PT = 128           # partitions per tile
KT = T // PT       # 4 T-tiles
NCORES = 8
CHUNK_COLS = 1536  # target input-chunk width (columns)
MCH = [(0, 128), (128, 128), (256, 128), (384, 16)]  # 128-aligned ROI chunks

_cache = {}


def _segment_structure(parc_weight):
    """Derive labels/counts and the sorted-by-length column permutation."""
    pw = np.asarray(parc_weight)
    labels = pw.argmax(axis=0)                       # [V]
    counts = np.bincount(labels, minlength=R)        # [R]
    # verify parc_weight really is the row-normalized one-hot we assume
    expect = np.zeros_like(pw)
    expect[labels, np.arange(V)] = (np.float32(1.0) / counts[labels].astype(np.float32))
    ok = bool(np.allclose(pw, expect, rtol=0, atol=1e-7)) and counts.min() > 0
    order = np.argsort(counts, kind="stable")        # ROI ids sorted by length
    roi_rank = np.empty(R, dtype=np.int64)
    roi_rank[order] = np.arange(R)
    perm = np.argsort(roi_rank[labels], kind="stable")   # voxel permutation
    sorted_counts = counts[order]
    # run-length encode sorted_counts -> groups of (L, n_g)
    groups = []
    i = 0
    while i < R:
        j = i
        while j < R and sorted_counts[j] == sorted_counts[i]:
            j += 1
        groups.append((int(sorted_counts[i]), j - i))
        i = j
    return ok, labels, counts, order, perm, groups


def _make_chunks(groups):
    """Pack groups into input-DMA chunks of ~CHUNK_COLS columns.

    The first chunk is kept small (fast pipeline start) and the tail of the
    column range is split into smaller chunks (short DVE tail after the last
    DMA lands). Returns a list of chunks; each chunk is (col_off, width,
    [(L, n_g, seg_col)]) where col_off/width are voxel-column offsets into
    the permuted imgs row and seg_col is the output column of the group's
    first ROI."""
    total = sum(L * n for L, n in groups)

    def build(target_fn):
        chunks = []
        cur = []
        cur_off = 0
        off = 0
        seg_col = 0
        for (L, n_g) in groups:
            cur.append((L, n_g, seg_col))
            off += L * n_g
            seg_col += n_g
            if off - cur_off >= target_fn(cur_off):
                chunks.append((cur_off, off - cur_off, cur))
                cur = []
                cur_off = off
        if cur:
            chunks.append((cur_off, off - cur_off, cur))
        return chunks

    uniform = build(lambda o: CHUNK_COLS)
    # tile 0: small first chunk so the DVE starts early
    head = build(lambda o: 768 if o == 0 else CHUNK_COLS)
    # last tile: taper the final chunks so the post-DMA DVE tail is short
    tail = build(lambda o: max(900, min(CHUNK_COLS, (total - o) // 2))
                 if total - o <= 2 * CHUNK_COLS else CHUNK_COLS)
    return [head] + [uniform] * (KT - 2) + [tail]


def _build_program(groups):
    import concourse.tile as tile
    from concourse import bacc, mybir

    F32 = mybir.dt.float32
    nc = bacc.Bacc("TRN2", target_bir_lowering=False, debug=False, num_devices=NCORES)
    x = nc.dram_tensor("x", [T, V], F32, kind="ExternalInput").ap()
    segsum = nc.dram_tensor("segsum", [T, R], F32, kind="ExternalOutput").ap()
    # packed upper blocks of the gram: columns = concat over m of row-block
    # m's upper part (widths 400/272/144/16); host unpacks
    PACKW = sum(R - moff for moff, _ in MCH)
    conn = nc.dram_tensor("conn", [PT, PACKW], F32, kind="ExternalOutput").ap()

    chunks_per_tile = _make_chunks(groups)
    max_w = max(w for tile_chunks in chunks_per_tile
                for _, w, _ in tile_chunks)

    with tile.TileContext(nc) as tc, ExitStack() as ctx:
        xin = ctx.enter_context(tc.tile_pool(name="xin", bufs=4))
        segp = ctx.enter_context(tc.tile_pool(name="seg", bufs=1))
        psum = ctx.enter_context(tc.tile_pool(name="psum", bufs=1, space="PSUM"))
        outp = ctx.enter_context(tc.tile_pool(name="out", bufs=4))

        conn_ps = [psum.tile([PT, R], F32, tag=f"conn{m}", name=f"conn_ps{m}")
                   for m in range(4)]
        seg_tiles = [segp.tile([PT, R], F32, tag=f"seg{k}", name=f"seg{k}")
                     for k in range(KT)]

        for k in range(KT):
            st = seg_tiles[k]
            for (coff, cw, cgroups) in chunks_per_tile[k]:
                xt = xin.tile([PT, max_w], F32, tag="xt", name=f"xt{k}_{coff}")
                nc.sync.dma_start(xt[:, :cw], x[k * PT:(k + 1) * PT,
                                                coff:coff + cw])
                loc = 0
                for (L, n_g, seg_col) in cgroups:
                    src = xt[:, loc:loc + n_g * L].rearrange(
                        "p (n l) -> p n l", l=L)
                    nc.vector.reduce_sum(st[:, seg_col:seg_col + n_g], src,
                                         axis=mybir.AxisListType.X)
                    loc += n_g * L
            nc.sync.dma_start(segsum[k * PT:(k + 1) * PT, :], st[:])
            # symmetric gram: only upper blocks (m <= s), s-outer so each
            # matmul issues as soon as the columns it reads are reduced.
            # Bank m is cleared by its first matmul (k==0, s==m); later
            # blocks overwrite-or-accumulate via per-element has_written.
            for (soff, sw) in MCH:
                for (moff, ms) in MCH:
                    if moff > soff:
                        continue
                    nc.tensor.matmul(conn_ps[moff // PT][:ms, soff:soff + sw],
                                     st[:, moff:moff + ms],
                                     st[:, soff:soff + sw],
                                     start=(k == 0 and moff == soff),
                                     stop=(k == KT - 1))
        ct = outp.tile([PT, PACKW], F32, tag="connout", name="connout")
        poff = 0
        for i, (moff, ms) in enumerate(MCH):
            uw = R - moff                       # valid (upper) width for row m
            src = conn_ps[moff // PT][:ms, moff:]
            dst = ct[:ms, poff:poff + uw]
            if i % 2 == 0:
                nc.scalar.mul(dst, src, 1.0)
            else:
                nc.vector.tensor_copy(dst, src)
            poff += uw
        nc.sync.dma_start(conn[:, :], ct[:, :])

    nc.compile()
    return nc


def _reference_fallback(imgs, parc_weight):
    """Pure-numpy reference path, used only if parc_weight is not one-hot."""
    lat = imgs.reshape(N, T, V).astype(np.float32) @ parc_weight.T.astype(np.float32)
    x = lat - lat.mean(axis=1, keepdims=True)
    x = x / (np.linalg.norm(x, axis=1, keepdims=True) + np.float32(EPS))
    conn = np.einsum("ntr,nts->nrs", x, x)
    row, col = np.triu_indices(R, k=1)
    return conn[:, row, col][:, None, :].astype(np.float32), lat.astype(np.float32)


def kernel(imgs, parc_weight):
    imgs = np.asarray(imgs)
    parc_weight = np.asarray(parc_weight, dtype=np.float32)
    ok, labels, counts, order, perm, groups = _segment_structure(parc_weight)
    if not ok:
        return _reference_fallback(imgs, parc_weight)

    key = tuple(groups)
    if key not in _cache:
        _cache[key] = _build_program(groups)
    nc = _cache[key]

    from concourse.bass_utils import run_bass_kernel_spmd

    imgs_flat = imgs.reshape(N, T, V)
    imgs_perm = np.take(imgs_flat, perm, axis=2)       # [8, 512, 16384]
    in_maps = [{"x": np.ascontiguousarray(imgs_perm[n])} for n in range(N)]
    res = run_bass_kernel_spmd(nc, in_maps, core_ids=list(range(NCORES)))

    segsum = np.stack([r["segsum"] for r in res.results])   # [8, 512, 400] sorted order
    connpack = np.stack([r["conn"] for r in res.results])   # [8, 128, 832] packed upper
    # unpack row-blocks, then mirror upper blocks into lower blocks
    conn_raw = np.empty((N, R, R), dtype=np.float32)
    poff = 0
    for (moff, ms) in MCH:
        uw = R - moff
        conn_raw[:, moff:moff + ms, moff:] = connpack[:, :ms, poff:poff + uw]
        poff += uw
    for (moff, ms) in MCH:
        for (soff, sw) in MCH:
            if soff > moff:
                conn_raw[:, soff:soff + sw, moff:moff + ms] = \
                    conn_raw[:, moff:moff + ms, soff:soff + sw].transpose(0, 2, 1)

    w = np.float32(1.0) / counts[order].astype(np.float32)  # [400]
    latent_s = segsum * w[None, None, :]
    latent = np.empty_like(latent_s)
    latent[:, :, order] = latent_s                          # unpermute ROIs

    # center on host in float64: conn_c = conn_raw - outer(sums, sums)/T
    sums = segsum.sum(axis=1, dtype=np.float64)             # [8, 400]
    conn_c = conn_raw.astype(np.float64) - \
        sums[:, :, None] * sums[:, None, :] * (1.0 / T)

    diag = conn_c[:, np.arange(R), np.arange(R)]            # [8, 400] = ||xc||^2
    nrm = np.sqrt(np.maximum(diag, 0.0)).astype(np.float32)
    f = w[None, :] / (w[None, :] * nrm + np.float32(EPS))   # [8, 400]
    conn_n = (conn_c * f[:, :, None].astype(np.float64)
              * f[:, None, :].astype(np.float64)).astype(np.float32)
    conn_full = np.empty_like(conn_n)
    conn_full[:, order[:, None], order[None, :]] = conn_n   # unpermute both axes

    row, col = np.triu_indices(R, k=1)
    conn_vec = conn_full[:, row, col]                       # [8, 79800]
    cls_token = conn_vec[:, None, :].astype(np.float32)
    return cls_token, latent.astype(np.float32)
